# revision 1
# baseline (speedup 1.0000x reference)
"""IndRNN kernel for 8 Trainium2 NeuronCores.

Math: h_t = relu(x_t @ W + b + u * h_{t-1}), h_0 = ones.  Output all h_t.

Strategy
--------
- Data-parallel over batch: B=32 -> 4 batches per core.
- Host prep: sort hidden units by u (ascending) and permute W's columns /
  b / u accordingly; pre-transpose x to [B, D, T] so the device matmul can
  consume it directly as the moving operand (contraction dim on partitions).
- Device per core: xw = W^T @ x^T produced as [h, t] tiles in PSUM (the
  layout the time-scan wants - no on-device transpose needed).
- Recurrence: for lanes with large u (sorted upper half) rewrite
      h_t = max(u*h_{t-1} + xw_t, 0)
  via g_i = h_{t0+i} * u^{-(i+1)} into the max-plus scan
      g_i = max(g_{i-1} + xw_{t0+i} * u^{-(i+1)}, 0)
  which maps onto the DVE tensor_tensor_scan (op0=add, op1=max).  Chunk
  length is capped by fp32 range of u^-L; chunks are chained by
  rebasing the state back to h-scale (g_last * u^L) between chunks.
- For lanes with small u (sorted lower half) the recurrence forgets
  quickly: use K fixed-point sweeps  h^{(k)} = relu(xw + u * shift(h^{(k-1)}))
  which is exact for influence windows <= K.
- Output written as [B, H_perm, T]; host inverse-permutes/transposes back.
"""

import sys

for _p in ("/opt/trn_rl_repo",):
    if _p not in sys.path:
        sys.path.insert(0, _p)

from contextlib import ExitStack

import numpy as np

import concourse.bass as bass
import concourse.tile as tile
from concourse import bacc, mybir
from concourse.bass_utils import run_bass_kernel_spmd

F32 = mybir.dt.float32
ALU = mybir.AluOpType
ACTF = mybir.ActivationFunctionType

B, T, D, H = 32, 4096, 256, 256
NCORES = 8
BLOC = B // NCORES  # batches per core


def _chunk_len(u, cap=512, limit=1.0e38, bound_scale=64.0):
    """Largest scan chunk length L with max_p bound_p * u_p^-L < limit."""
    u = np.asarray(u, np.float64)
    usafe = np.minimum(u, 0.9995)
    bound = bound_scale / np.sqrt(np.maximum(1.0 - usafe**2, 1e-6))
    L = 1
    while L < cap and float(np.max(bound * u ** (-(L + 1.0)))) < limit:
        L += 1
    return L


def _grids(t_total, l1):
    """Chunk grid for the scan half and a shared matmul N grid.

    Matmul tiles are multiples of l1 (<=512 fp32 cap) so every chunk lies
    inside one matmul tile; the tail tile holds the ragged chunks.
    """
    per = max(1, min(4, 512 // l1)) * l1
    chunks = []
    t0 = 0
    while t0 < t_total:
        lc = min(l1, t_total - t0)
        chunks.append((t0, lc))
        t0 += lc
    ngrid = []
    t0 = 0
    while t0 + per <= (t_total // l1) * l1:
        ngrid.append((t0, per))
        t0 += per
    if t0 < t_total:
        ngrid.append((t0, t_total - t0))
    return ngrid, chunks


def _build(nc, t_total, l1, k0, bloc):
    """Emit the per-core program.  Returns nothing; mutates nc."""
    ngrid, chunks1 = _grids(t_total, l1)

    xt_d = nc.declare_dram_parameter("xt", [bloc, D, t_total], F32, isOutput=False)
    w_d = nc.declare_dram_parameter("w", [D, H], F32, isOutput=False)
    bc_d = nc.declare_dram_parameter("bcol", [H, 1], F32, isOutput=False)
    uc_d = nc.declare_dram_parameter("ucol", [H, 1], F32, isOutput=False)
    uneg_d = nc.declare_dram_parameter("uneg", [128, l1], F32, isOutput=False)
    upos_d = nc.declare_dram_parameter("upos", [128, l1], F32, isOutput=False)
    ulast_d = nc.declare_dram_parameter("ulast", [128, 1], F32, isOutput=False)
    out_d = nc.declare_dram_parameter("out", [bloc, H, t_total], F32, isOutput=True)

    with tile.TileContext(nc) as tc, ExitStack() as ctx:
        const = ctx.enter_context(tc.tile_pool(name="const", bufs=1))
        xt_pool = ctx.enter_context(tc.tile_pool(name="xt", bufs=3))
        psum_pool = ctx.enter_context(
            tc.tile_pool(name="psum", bufs=3, space=bass.MemorySpace.PSUM)
        )
        xw0_pool = ctx.enter_context(tc.tile_pool(name="xw0", bufs=2))
        d0_pool = ctx.enter_context(tc.tile_pool(name="d0", bufs=2))
        g_pool = ctx.enter_context(tc.tile_pool(name="g", bufs=2))
        h1_pool = ctx.enter_context(tc.tile_pool(name="h1", bufs=2))
        win_pool = ctx.enter_context(tc.tile_pool(name="win", bufs=3))
        s_pool = ctx.enter_context(tc.tile_pool(name="s", bufs=1))
        init_pool = ctx.enter_context(tc.tile_pool(name="init", bufs=8))

        # persistent weights / tables
        w_sb = []
        for dh in range(2):
            wt = const.tile([128, H], F32, tag=f"w{dh}")
            nc.sync.dma_start(wt[:, :], w_d[dh * 128 : (dh + 1) * 128, :])
            w_sb.append(wt)
        uneg_sb = const.tile([128, l1], F32, tag="uneg")
        nc.sync.dma_start(uneg_sb[:, :], uneg_d[:, :])
        upos_sb = const.tile([128, l1], F32, tag="upos")
        nc.sync.dma_start(upos_sb[:, :], upos_d[:, :])
        ulast_sb = const.tile([128, 1], F32, tag="ulast")
        nc.sync.dma_start(ulast_sb[:, :], ulast_d[:, :])
        ucol0_sb = const.tile([128, 1], F32, tag="ucol0")
        nc.sync.dma_start(ucol0_sb[:, :], uc_d[0:128, :])
        bcol0_sb = const.tile([128, 1], F32, tag="bcol0")
        nc.sync.dma_start(bcol0_sb[:, :], bc_d[0:128, :])
        bcol1_sb = const.tile([128, 1], F32, tag="bcol1")
        nc.sync.dma_start(bcol1_sb[:, :], bc_d[128:256, :])
        zeros_sb = const.tile([128, l1], F32, tag="zeros")
        nc.vector.memset(zeros_sb[:, :], 0.0)

        n_full = sum(1 for _, lc in chunks1 if lc == l1)
        full_end = n_full * l1

        for b in range(bloc):
            xw0 = xw0_pool.tile([128, t_total], F32, tag="xw0")
            d0 = d0_pool.tile([128, t_total], F32, tag="d0")
            g = g_pool.tile([128, t_total], F32, tag="g")
            h1 = h1_pool.tile([128, t_total], F32, tag="h1")

            # ---- matmul: xw tiles [h, t] in PSUM, then fanout ----
            for t0, nt in ngrid:
                xts = []
                for dh in range(2):
                    xtt = xt_pool.tile([128, nt], F32, tag=f"xt{dh}")
                    nc.sync.dma_start(
                        xtt[:, :], xt_d[b, dh * 128 : (dh + 1) * 128, t0 : t0 + nt]
                    )
                    xts.append(xtt)
                ps0 = psum_pool.tile([128, nt], F32, tag="ps0")
                ps1 = psum_pool.tile([128, nt], F32, tag="ps1")
                for dh in range(2):
                    nc.tensor.matmul(
                        ps0[:, :],
                        w_sb[dh][:, 0:128],
                        xts[dh][:, :],
                        start=(dh == 0),
                        stop=(dh == 1),
                    )
                for dh in range(2):
                    nc.tensor.matmul(
                        ps1[:, :],
                        w_sb[dh][:, 128:256],
                        xts[dh][:, :],
                        start=(dh == 0),
                        stop=(dh == 1),
                    )
                # small-u half: copy xw to SBUF for the window sweeps
                nc.scalar.activation(xw0[:, t0 : t0 + nt], ps0[:, :], ACTF.Copy)
                # large-u half: premultiply by u^-(i+1) (bias folded in)
                off = 0
                while off < nt:
                    # run of whole chunks with a common length
                    lc = min(l1, t_total - (t0 + off))
                    n = 1
                    if lc == l1:
                        n = min((nt - off) // l1, (full_end - (t0 + off)) // l1)
                        n = max(n, 1)
                    span = n * lc
                    in0 = ps1[:, off : off + span].rearrange("p (n l) -> p n l", l=lc)
                    out = d0[:, t0 + off : t0 + off + span].rearrange(
                        "p (n l) -> p n l", l=lc
                    )
                    tab = uneg_sb[:, 0:lc].unsqueeze(1).broadcast_to([128, n, lc])
                    nc.vector.scalar_tensor_tensor(
                        out, in0, bcol1_sb[:, :], tab, op0=ALU.add, op1=ALU.mult
                    )
                    off += span

            # ---- small-u half (h 0:128): K fixed-point sweeps ----
            hcur = win_pool.tile([128, t_total + 1], F32, tag="win")
            nc.vector.memset(hcur[:, 0:1], 1.0)
            nc.scalar.activation(
                hcur[:, 1 : t_total + 1], xw0[:, :], ACTF.Relu, bias=bcol0_sb[:, :]
            )
            for _k in range(1, k0):
                s = s_pool.tile([128, t_total], F32, tag="s")
                nc.vector.scalar_tensor_tensor(
                    s[:, :],
                    hcur[:, 0:t_total],
                    ucol0_sb[:, :],
                    xw0[:, :],
                    op0=ALU.mult,
                    op1=ALU.add,
                )
                hnew = win_pool.tile([128, t_total + 1], F32, tag="win")
                nc.vector.memset(hnew[:, 0:1], 1.0)
                nc.scalar.activation(
                    hnew[:, 1 : t_total + 1], s[:, :], ACTF.Relu, bias=bcol0_sb[:, :]
                )
                hcur = hnew
            nc.sync.dma_start(out_d[b, 0:128, :], hcur[:, 1 : t_total + 1])

            # ---- large-u half (h 128:256): chained rescaled scans ----
            init_ap = None
            for ci, (c0, lc) in enumerate(chunks1):
                initial = 1.0 if ci == 0 else init_ap
                nc.vector.tensor_tensor_scan(
                    g[:, c0 : c0 + lc],
                    d0[:, c0 : c0 + lc],
                    zeros_sb[:, 0:lc],
                    initial,
                    op0=ALU.add,
                    op1=ALU.max,
                )
                if ci < len(chunks1) - 1:
                    it = init_pool.tile([128, 1], F32, tag="it")
                    nc.vector.tensor_scalar(
                        it[:, :],
                        g[:, c0 + lc - 1 : c0 + lc],
                        ulast_sb[:, :],
                        None,
                        op0=ALU.mult,
                    )
                    init_ap = it[:, :]
            # postmultiply g -> h (bulk on gpsimd, off the critical chain)
            if full_end > 0:
                nc.gpsimd.tensor_tensor(
                    h1[:, 0:full_end].rearrange("p (n l) -> p n l", l=l1),
                    g[:, 0:full_end].rearrange("p (n l) -> p n l", l=l1),
                    upos_sb[:, :].unsqueeze(1).broadcast_to([128, n_full, l1]),
                    op=ALU.mult,
                )
            if full_end < t_total:
                rem = t_total - full_end
                nc.gpsimd.tensor_tensor(
                    h1[:, full_end:t_total],
                    g[:, full_end:t_total],
                    upos_sb[:, 0:rem],
                    op=ALU.mult,
                )
            nc.sync.dma_start(out_d[b, 128:256, :], h1[:, :])


def _host_prep(x, W, b, u, t_total=T):
    """Sort/permute/transpose on host; build tables.  Returns (inmaps, perm)."""
    x = np.ascontiguousarray(np.asarray(x, np.float32))
    W = np.asarray(W, np.float32)
    b = np.asarray(b, np.float32)
    u = np.asarray(u, np.float32)

    perm = np.argsort(u, kind="stable")
    u_s = u[perm]
    W_p = np.ascontiguousarray(W[:, perm], np.float32)
    b_p = np.ascontiguousarray(b[perm], np.float32)

    u1 = u_s[128:].astype(np.float64)
    l1 = _chunk_len(u1)
    i = np.arange(1, l1 + 1, dtype=np.float64)
    uneg = (u1[:, None] ** (-i[None, :])).astype(np.float32)
    upos = (u1[:, None] ** (i[None, :])).astype(np.float32)
    ulast = (u1**l1).astype(np.float32)[:, None]
    assert np.all(np.isfinite(uneg))

    xt = np.ascontiguousarray(np.swapaxes(x, 1, 2))  # [B, D, T]
    bloc = x.shape[0] // NCORES if x.shape[0] >= NCORES else x.shape[0]

    common = {
        "w": W_p,
        "bcol": np.ascontiguousarray(b_p[:, None]),
        "ucol": np.ascontiguousarray(u_s[:, None]),
        "uneg": uneg,
        "upos": upos,
        "ulast": ulast,
    }
    in_maps = []
    ncores = x.shape[0] // bloc
    for c in range(ncores):
        m = dict(common)
        m["xt"] = np.ascontiguousarray(xt[c * bloc : (c + 1) * bloc])
        in_maps.append(m)
    return in_maps, perm, l1, bloc


# window sweep count for the small-u half: err ~ u_max0^(K+1); u_max0 <= 0.5
K0 = 12

# set by test harnesses to profile: kernel() stores the raw results here
LAST_RESULT = None


def kernel(x, W, b, u):
    global LAST_RESULT
    import os

    in_maps, perm, l1, bloc = _host_prep(x, W, b, u)
    ncores = len(in_maps)

    nc = bacc.Bacc("TRN2", target_bir_lowering=False, debug=False)
    _build(nc, T, l1, K0, bloc)
    nc.compile()

    trace = bool(os.environ.get("INDRNN_TRACE"))
    res = run_bass_kernel_spmd(
        nc, in_maps, core_ids=list(range(ncores)), trace=trace
    )
    LAST_RESULT = res
    out_dev = np.concatenate([r["out"] for r in res.results], axis=0)  # [B,H,T]

    out = np.empty((x.shape[0], T, H), np.float32)
    out[:, :, perm] = np.swapaxes(out_dev, 1, 2)
    return out

